# revision 7
# baseline (speedup 1.0000x reference)
"""Trainium2 Bass kernel for ConvTranspose3dMPS (FFT-based reference).

Math: the reference computes out = ifftn(fftn(upsample(x)) * fftn(pad(w))).real
summed over in-channels, i.e. a *circular* 3x3x3 convolution of the 2x
repeat-upsampled input.  Polyphase-decomposing by output parity (pz,py,px)
collapses it to eight 2x2x2-tap circular convolutions on the original 16^3
grid with parity-combined weights:

    out[b,co, 2m+p] = sum_{ci, t in {0,1}^3} We[p,t,co,ci] * x[b,ci,(m-t) mod 16]

where (per axis)  We uses  A[p=0] = [[1,0,0],[0,1,1]],  A[p=1] = [[1,1,0],[0,0,1]]
applied to the 3 kernel taps.  That is 64 matmuls of [K=128,M=128]x[128,256]
per (batch, z-half) shard — a perfect fit for the 128x128 PE array.

Sharding: 8 cores = batch(4) x z-half(2).  Each core receives:
  xh2 [128, 2890] f32 : halo-padded x slab (ci, 9x17x17 flat), duplicated on
                        partitions 64..127 at +289 elements so one access
                        pattern reads tap tz=0 (lower) and tz=1 (upper).
  wt  [128, 1024] f32 : parity-combined weights, [k=(tz,ci), (g,q)*128+m],
                        m = 32*(2*py+px) + co, g = pz, q = (ty,tx).
and returns out [128, 4096] f32 : [32*(2py+px)+co, (pz*8+mz)*256 + my*16 + mx].
"""

import numpy as np

P = 128
B, CI, CO, NZ = 4, 64, 32, 16
ZPL = 17 * 17            # elements per halo z-plane
XH_LEN = 9 * ZPL         # 2601
XH2_W = XH_LEN + ZPL     # 2890 (upper-partition copy shifted +289)
WT_W = 8 * 128           # 1024
OUT_W = 2 * 8 * 256      # 4096
N_CORES = 8

_CACHE = {}


def _build_program():
    import concourse.bacc as bacc
    import concourse.tile as tile
    import concourse.mybir as mybir

    f32 = mybir.dt.float32
    f32r = mybir.dt.float32r

    nc = bacc.Bacc("TRN2", target_bir_lowering=False, debug=False)
    xh2_d = nc.declare_dram_parameter("xh2", [P, XH2_W], f32r, isOutput=False)
    wt_d = nc.declare_dram_parameter("wt", [P, WT_W], f32r, isOutput=False)
    out_d = nc.declare_dram_parameter("out", [P, OUT_W], f32, isOutput=True)

    with tile.TileContext(nc) as tc:
        with (
            tc.tile_pool(name="io", bufs=1) as io_pool,
            tc.tile_pool(name="ps", bufs=8, space="PSUM") as ps_pool,
        ):
            xh2 = io_pool.tile([P, XH2_W], f32r, tag="xh2")
            wt = io_pool.tile([P, WT_W], f32r, tag="wt")
            ob = io_pool.tile([P, OUT_W], f32, tag="ob")
            nc.sync.dma_start(wt[:], wt_d[:])
            nc.sync.dma_start(xh2[:], xh2_d[:])

            for g in range(2):
                psums = [
                    ps_pool.tile([P, 256], f32, tag="ps", name=f"ps_{g}_{mz}")
                    for mz in range(8)
                ]
                for q in range(4):
                    ty, tx = q >> 1, q & 1
                    lhsT = wt[:, (g * 4 + q) * 128:(g * 4 + q + 1) * 128]
                    for mz in range(8):
                        base = (mz + 1) * ZPL + (1 - ty) * 17 + (1 - tx)
                        rhs = (
                            xh2[:, base:base + 272]
                            .rearrange("p (a b) -> p a b", a=16, b=17)[:, :, 0:16]
                        )
                        nc.tensor.matmul(
                            psums[mz][:], lhsT, rhs,
                            start=(q == 0), stop=(q == 3),
                        )
                for mz in range(8):
                    dst = ob[:, (g * 8 + mz) * 256:(g * 8 + mz + 1) * 256]
                    if mz % 3 == 2:
                        nc.scalar.copy(dst, psums[mz][:])
                    else:
                        nc.vector.tensor_copy(dst, psums[mz][:])

            nc.sync.dma_start(out_d[:], ob[:])

    nc.compile()
    return nc


def _get_program():
    if "nc" not in _CACHE:
        _CACHE["nc"] = _build_program()
    return _CACHE["nc"]


# Per-axis tap combination: A[p][t, d]
_A = np.array(
    [[[1, 0, 0], [0, 1, 1]],
     [[1, 1, 0], [0, 0, 1]]], dtype=np.float32)


def _prep_weights(weight: np.ndarray) -> np.ndarray:
    # We[pz,py,px,tz,ty,tx,co,ci]
    We = np.einsum("ptd,que,rvf,oidef->pqrtuvoi", _A, _A, _A,
                   weight.astype(np.float32), optimize=True)
    WT = np.zeros((2, 4, P, P), np.float32)
    for g in range(2):
        for q in range(4):
            ty, tx = q >> 1, q & 1
            for tz in range(2):
                for pp in range(4):
                    py, px = pp >> 1, pp & 1
                    WT[g, q, 64 * tz:64 * tz + 64, 32 * pp:32 * pp + 32] = (
                        We[g, py, px, tz, ty, tx].T
                    )
    # DRAM layout [k, (g,q)*128 + m]
    return np.ascontiguousarray(WT.transpose(2, 0, 1, 3).reshape(P, WT_W))


def _prep_x_shard(xb: np.ndarray, zhalf: int) -> np.ndarray:
    # xb: [CI, 16, 16, 16] one batch; build halo slab + partition duplication
    z0 = 8 * zhalf
    zidx = (z0 - 1 + np.arange(9)) % 16
    yidx = (np.arange(17) - 1) % 16
    xh = xb[:, zidx][:, :, yidx][:, :, :, yidx].reshape(CI, XH_LEN)
    buf = np.zeros((P, XH2_W), np.float32)
    buf[:CI, :XH_LEN] = xh
    buf[CI:, ZPL:ZPL + XH_LEN] = xh
    return buf


def _unshard(results) -> np.ndarray:
    out = np.empty((B, CO, 32, 32, 32), np.float32)
    for core in range(N_CORES):
        b, zhalf = core // 2, core % 2
        v = results[core]["out"].reshape(2, 2, CO, 2, 8, 16, 16)
        # indices: [py, px, co, pz, mz, my, mx] -> [co, (mz,pz), (my,py), (mx,px)]
        blk = v.transpose(2, 4, 3, 5, 0, 6, 1).reshape(CO, 16, 32, 32)
        out[b, :, 16 * zhalf:16 * zhalf + 16] = blk
    return out


def kernel(x: np.ndarray, weight: np.ndarray) -> np.ndarray:
    from concourse.bass_utils import run_bass_kernel_spmd

    x = np.ascontiguousarray(np.asarray(x), dtype=np.float32)
    weight = np.ascontiguousarray(np.asarray(weight), dtype=np.float32)

    wt_host = _prep_weights(weight)
    in_maps = []
    for core in range(N_CORES):
        b, zhalf = core // 2, core % 2
        in_maps.append({"xh2": _prep_x_shard(x[b], zhalf), "wt": wt_host})

    nc = _get_program()
    res = run_bass_kernel_spmd(nc, in_maps, core_ids=list(range(N_CORES)))
    return _unshard(res.results)


# revision 10
# speedup vs baseline: 1.1216x; 1.1216x over previous
"""Trainium2 Bass kernel for ConvTranspose3dMPS (FFT-based reference).

Math: the reference computes out = ifftn(fftn(upsample(x)) * fftn(pad(w))).real
summed over in-channels, i.e. a *circular* 3x3x3 convolution of the 2x
repeat-upsampled input.  Polyphase-decomposing by output parity (pz,py,px)
collapses it to eight 2x2x2-tap circular convolutions on the original 16^3
grid with parity-combined weights:

    out[b,co, 2m+p] = sum_{ci, t in {0,1}^3} We[p,t,co,ci] * x[b,ci,(m-t) mod 16]

where (per axis)  We uses  A[p=0] = [[1,0,0],[0,1,1]],  A[p=1] = [[1,1,0],[0,0,1]]
applied to the 3 kernel taps.  That is 32 matmuls of [K=128,M=128]x[128,512]
per (batch, z-half) shard — a perfect fit for the 128x128 PE array.

Sharding: 8 cores = batch(4) x z-half(2).  Each core receives:
  xh2 [128, 2890] f32 : halo-padded x slab (ci, 9x17x17 flat), duplicated on
                        partitions 64..127 at +289 elements so one access
                        pattern reads tap tz=0 (lower) and tz=1 (upper).
  wt  [128, 1024] f32 : parity-combined weights, [k=(tz,ci), (g,q)*128+m],
                        m = 32*(2*py+px) + co, g = pz, q = (ty,tx).
returns out [128, 4096] f32 : [32*(2py+px)+co, pair*1024 + pz*512 + mzp*256
                              + my*16 + mx]   (mz = 2*pair + mzp).

Kernel structure (per core): weights DMA (ACT ring) + x in 2 column-chunks
(SP ring); 8 warm-up matmuls on the weight tile ramp the PE HAM clock while
x streams in; 32 f32r matmuls (4 accumulating per PSUM bank) with 3-free-dim
strided rhs APs; PSUM evacuated by DVE/ACT copies; output DMAed in 2 chunks.
"""

import numpy as np

P = 128
B, CI, CO, NZ = 4, 64, 32, 16
ZPL = 17 * 17            # elements per halo z-plane
XH_LEN = 9 * ZPL         # 2601
XH2_W = XH_LEN + ZPL     # 2890 (upper-partition copy shifted +289)
WT_W = 8 * 128           # 1024
OUT_W = 2 * 8 * 256      # 4096
XCHUNK = 5 * ZPL         # 1445: column split point between mz pairs 0-1 / 2-3
N_CORES = 8
N_WARMUP = 8

_CACHE = {}


def _build_program():
    import concourse.bacc as bacc
    import concourse.tile as tile
    import concourse.mybir as mybir

    f32 = mybir.dt.float32
    f32r = mybir.dt.float32r

    nc = bacc.Bacc("TRN2", target_bir_lowering=False, debug=False)
    xh2_d = nc.declare_dram_parameter("xh2", [P, XH2_W], f32r, isOutput=False)
    wt_d = nc.declare_dram_parameter("wt", [P, WT_W], f32r, isOutput=False)
    out_d = nc.declare_dram_parameter("out", [P, OUT_W], f32, isOutput=True)

    with tile.TileContext(nc) as tc:
        with (
            tc.tile_pool(name="io", bufs=1) as io_pool,
            tc.tile_pool(name="ps", bufs=8, space="PSUM") as ps_pool,
        ):
            xh2 = io_pool.tile([P, XH2_W], f32r, tag="xh2")
            wt = io_pool.tile([P, WT_W], f32r, tag="wt")
            ob = io_pool.tile([P, OUT_W], f32, tag="ob")
            # weights on the ACT HWDGE ring; x chunks on the SP ring
            nc.scalar.dma_start(wt[:], wt_d[:])
            nc.sync.dma_start(xh2[:, 0:XCHUNK], xh2_d[:, 0:XCHUNK])
            nc.sync.dma_start(xh2[:, XCHUNK:XH2_W], xh2_d[:, XCHUNK:XH2_W])

            # PE warm-up on the weight tile (only needs wt; runs during x DMA)
            wu = ps_pool.tile([P, 512], f32, tag="wu", name="wu", bufs=1)
            for i in range(N_WARMUP):
                nc.tensor.matmul(wu[:], wt[:, 0:128], wt[:, 0:512],
                                 start=True, stop=True)

            def rhs_ap(pair, ty, tx):
                base = (2 * pair + 1) * ZPL + (1 - ty) * 17 + (1 - tx)
                return (
                    xh2[:, base:base + 578]
                    .rearrange("p (a b) -> p a b", a=2, b=289)[:, :, 0:272]
                    .rearrange("p a (c d) -> p a c d", c=16, d=17)[:, :, :, 0:16]
                )

            n_evac = 0
            for blk in range(2):               # mz pairs [2*blk, 2*blk+1]
                psums = [
                    ps_pool.tile([P, 512], f32, tag="ps", name=f"ps_{blk}_{i}", bufs=7)
                    for i in range(4)          # (pair_in_blk, g)
                ]
                for q in range(4):
                    ty, tx = q >> 1, q & 1
                    for g in range(2):
                        lhsT = wt[:, (g * 4 + q) * 128:(g * 4 + q + 1) * 128]
                        for pb in range(2):
                            pair = 2 * blk + pb
                            nc.tensor.matmul(
                                psums[2 * pb + g][:], lhsT, rhs_ap(pair, ty, tx),
                                start=(q == 0), stop=(q == 3),
                            )
                for pb in range(2):
                    for g in range(2):
                        pair = 2 * blk + pb
                        dst = ob[:, pair * 1024 + g * 512:pair * 1024 + g * 512 + 512]
                        if n_evac % 4 == 3:
                            nc.scalar.copy(dst, psums[2 * pb + g][:])
                        else:
                            nc.vector.tensor_copy(dst, psums[2 * pb + g][:])
                        n_evac += 1
                half = ob[:, blk * 2048:(blk + 1) * 2048]
                half_d = out_d[:, blk * 2048:(blk + 1) * 2048]
                if blk == 0:
                    nc.scalar.dma_start(half_d, half)
                else:
                    nc.sync.dma_start(half_d, half)

    nc.compile()
    return nc


def _get_program():
    if "nc" not in _CACHE:
        _CACHE["nc"] = _build_program()
    return _CACHE["nc"]


# Per-axis tap combination: A[p][t, d]
_A = np.array(
    [[[1, 0, 0], [0, 1, 1]],
     [[1, 1, 0], [0, 0, 1]]], dtype=np.float32)


def _prep_weights(weight: np.ndarray) -> np.ndarray:
    # We[pz,py,px,tz,ty,tx,co,ci]
    We = np.einsum("ptd,que,rvf,oidef->pqrtuvoi", _A, _A, _A,
                   weight.astype(np.float32), optimize=True)
    WT = np.zeros((2, 4, P, P), np.float32)
    for g in range(2):
        for q in range(4):
            ty, tx = q >> 1, q & 1
            for tz in range(2):
                for pp in range(4):
                    py, px = pp >> 1, pp & 1
                    WT[g, q, 64 * tz:64 * tz + 64, 32 * pp:32 * pp + 32] = (
                        We[g, py, px, tz, ty, tx].T
                    )
    # DRAM layout [k, (g,q)*128 + m]
    return np.ascontiguousarray(WT.transpose(2, 0, 1, 3).reshape(P, WT_W))


def _prep_x_shard(xb: np.ndarray, zhalf: int) -> np.ndarray:
    # xb: [CI, 16, 16, 16] one batch; build halo slab + partition duplication
    z0 = 8 * zhalf
    zidx = (z0 - 1 + np.arange(9)) % 16
    yidx = (np.arange(17) - 1) % 16
    xh = xb[:, zidx][:, :, yidx][:, :, :, yidx].reshape(CI, XH_LEN)
    buf = np.zeros((P, XH2_W), np.float32)
    buf[:CI, :XH_LEN] = xh
    buf[CI:, ZPL:ZPL + XH_LEN] = xh
    return buf


def _unshard(results) -> np.ndarray:
    out = np.empty((B, CO, 32, 32, 32), np.float32)
    for core in range(N_CORES):
        b, zhalf = core // 2, core % 2
        # free dims: (pair, pz, mzp, my, mx); partitions: (py, px, co)
        v = results[core]["out"].reshape(2, 2, CO, 4, 2, 2, 16, 16)
        # -> [co, (pair,mzp,pz), (my,py), (mx,px)]
        blk = v.transpose(2, 3, 5, 4, 6, 0, 7, 1).reshape(CO, 16, 32, 32)
        out[b, :, 16 * zhalf:16 * zhalf + 16] = blk
    return out


def kernel(x: np.ndarray, weight: np.ndarray) -> np.ndarray:
    import time
    from concourse.bass_utils import run_bass_kernel_spmd

    x = np.ascontiguousarray(np.asarray(x), dtype=np.float32)
    weight = np.ascontiguousarray(np.asarray(weight), dtype=np.float32)

    wt_host = _prep_weights(weight)
    in_maps = []
    for core in range(N_CORES):
        b, zhalf = core // 2, core % 2
        in_maps.append({"xh2": _prep_x_shard(x[b], zhalf), "wt": wt_host})

    nc = _get_program()
    last_exc = None
    for attempt in range(3):
        try:
            res = run_bass_kernel_spmd(nc, in_maps, core_ids=list(range(N_CORES)))
            return _unshard(res.results)
        except Exception as ex:  # intermittent NRT device errors: retry
            last_exc = ex
            time.sleep(2.0)
    raise last_exc


# revision 11
# speedup vs baseline: 1.1881x; 1.0592x over previous
"""Trainium2 Bass kernel for ConvTranspose3dMPS (FFT-based reference).

Math: the reference computes out = ifftn(fftn(upsample(x)) * fftn(pad(w))).real
summed over in-channels, i.e. a *circular* 3x3x3 convolution of the 2x
repeat-upsampled input.  Polyphase-decomposing by output parity (pz,py,px)
collapses it to eight 2x2x2-tap circular convolutions on the original 16^3
grid with parity-combined weights:

    out[b,co, 2m+p] = sum_{ci, t in {0,1}^3} We[p,t,co,ci] * x[b,ci,(m-t) mod 16]

where (per axis)  We uses  A[p=0] = [[1,0,0],[0,1,1]],  A[p=1] = [[1,1,0],[0,0,1]]
applied to the 3 kernel taps.  That is 32 matmuls of [K=128,M=128]x[128,512]
per (batch, z-half) shard — a perfect fit for the 128x128 PE array.

Sharding: 8 cores = batch(4) x z-half(2).  Each core receives:
  xh2 [128, 2890] f32 : halo-padded x slab (ci, 9x17x17 flat), duplicated on
                        partitions 64..127 at +289 elements so one access
                        pattern reads tap tz=0 (lower) and tz=1 (upper).
  wt  [128, 1024] f32 : parity-combined weights, [k=(tz,ci), (g,q)*128+m],
                        m = 32*(2*py+px) + co, g = pz, q = (ty,tx).
returns out [128, 4096] f32 : [32*(2py+px)+co, pair*1024 + pz*512 + mzp*256
                              + my*16 + mx]   (mz = 2*pair + mzp).

Kernel structure (per core): weights DMA (ACT ring) + x in 2 column-chunks
(SP ring); 8 warm-up matmuls on the weight tile ramp the PE HAM clock while
x streams in; 32 f32r matmuls (4 accumulating per PSUM bank) with 3-free-dim
strided rhs APs; PSUM evacuated by DVE/ACT copies; output DMAed in 2 chunks.
"""

import numpy as np

P = 128
B, CI, CO, NZ = 4, 64, 32, 16
ZPL = 17 * 17            # elements per halo z-plane
XH_LEN = 9 * ZPL         # 2601
XH2_W = XH_LEN + ZPL     # 2890 (upper-partition copy shifted +289)
WT_W = 8 * 128           # 1024
OUT_W = 2 * 8 * 256      # 4096
XCHUNK = 5 * ZPL         # 1445: column split point between mz pairs 0-1 / 2-3
N_CORES = 8
N_WARMUP = 8

_CACHE = {}


def _build_program():
    import concourse.bacc as bacc
    import concourse.tile as tile
    import concourse.mybir as mybir

    f32 = mybir.dt.float32
    f32r = mybir.dt.float32r

    nc = bacc.Bacc("TRN2", target_bir_lowering=False, debug=False)
    xh2_d = nc.declare_dram_parameter("xh2", [P, XH2_W], f32r, isOutput=False)
    wt_d = nc.declare_dram_parameter("wt", [P, WT_W], f32r, isOutput=False)
    out_d = nc.declare_dram_parameter("out", [P, OUT_W], f32, isOutput=True)

    with tile.TileContext(nc) as tc:
        with (
            tc.tile_pool(name="io", bufs=1) as io_pool,
            tc.tile_pool(name="ps", bufs=8, space="PSUM") as ps_pool,
        ):
            xh2 = io_pool.tile([P, XH2_W], f32r, tag="xh2")
            wt = io_pool.tile([P, WT_W], f32r, tag="wt")
            ob = io_pool.tile([P, OUT_W], f32, tag="ob")
            # all input DMAs on the SP HWDGE ring: FIFO per-engine ordering
            # guarantees wt lands first, then x chunk 1, then chunk 2
            nc.sync.dma_start(wt[:], wt_d[:])
            nc.sync.dma_start(xh2[:, 0:XCHUNK], xh2_d[:, 0:XCHUNK])
            nc.sync.dma_start(xh2[:, XCHUNK:XH2_W], xh2_d[:, XCHUNK:XH2_W])

            # PE warm-up on the weight tile (only needs wt; runs during x DMA)
            wu = ps_pool.tile([P, 512], f32, tag="wu", name="wu", bufs=1)
            for i in range(N_WARMUP):
                nc.tensor.matmul(wu[:], wt[:, 0:128], wt[:, 0:512],
                                 start=True, stop=True)

            def rhs_ap(pair, ty, tx):
                base = (2 * pair + 1) * ZPL + (1 - ty) * 17 + (1 - tx)
                return (
                    xh2[:, base:base + 578]
                    .rearrange("p (a b) -> p a b", a=2, b=289)[:, :, 0:272]
                    .rearrange("p a (c d) -> p a c d", c=16, d=17)[:, :, :, 0:16]
                )

            n_evac = 0
            for blk in range(2):               # mz pairs [2*blk, 2*blk+1]
                psums = [
                    ps_pool.tile([P, 512], f32, tag="ps", name=f"ps_{blk}_{i}", bufs=7)
                    for i in range(4)          # (pair_in_blk, g)
                ]
                for q in range(4):
                    ty, tx = q >> 1, q & 1
                    for g in range(2):
                        lhsT = wt[:, (g * 4 + q) * 128:(g * 4 + q + 1) * 128]
                        for pb in range(2):
                            pair = 2 * blk + pb
                            nc.tensor.matmul(
                                psums[2 * pb + g][:], lhsT, rhs_ap(pair, ty, tx),
                                start=(q == 0), stop=(q == 3),
                            )
                for pb in range(2):
                    for g in range(2):
                        pair = 2 * blk + pb
                        dst = ob[:, pair * 1024 + g * 512:pair * 1024 + g * 512 + 512]
                        if n_evac % 4 == 3:
                            nc.scalar.copy(dst, psums[2 * pb + g][:])
                        else:
                            nc.vector.tensor_copy(dst, psums[2 * pb + g][:])
                        n_evac += 1
                half = ob[:, blk * 2048:(blk + 1) * 2048]
                half_d = out_d[:, blk * 2048:(blk + 1) * 2048]
                if blk == 0:
                    nc.scalar.dma_start(half_d, half)
                else:
                    nc.sync.dma_start(half_d, half)

    nc.compile()
    return nc


def _get_program():
    if "nc" not in _CACHE:
        _CACHE["nc"] = _build_program()
    return _CACHE["nc"]


# Per-axis tap combination: A[p][t, d]
_A = np.array(
    [[[1, 0, 0], [0, 1, 1]],
     [[1, 1, 0], [0, 0, 1]]], dtype=np.float32)


def _prep_weights(weight: np.ndarray) -> np.ndarray:
    # We[pz,py,px,tz,ty,tx,co,ci]
    We = np.einsum("ptd,que,rvf,oidef->pqrtuvoi", _A, _A, _A,
                   weight.astype(np.float32), optimize=True)
    WT = np.zeros((2, 4, P, P), np.float32)
    for g in range(2):
        for q in range(4):
            ty, tx = q >> 1, q & 1
            for tz in range(2):
                for pp in range(4):
                    py, px = pp >> 1, pp & 1
                    WT[g, q, 64 * tz:64 * tz + 64, 32 * pp:32 * pp + 32] = (
                        We[g, py, px, tz, ty, tx].T
                    )
    # DRAM layout [k, (g,q)*128 + m]
    return np.ascontiguousarray(WT.transpose(2, 0, 1, 3).reshape(P, WT_W))


def _prep_x_shard(xb: np.ndarray, zhalf: int) -> np.ndarray:
    # xb: [CI, 16, 16, 16] one batch; build halo slab + partition duplication
    z0 = 8 * zhalf
    zidx = (z0 - 1 + np.arange(9)) % 16
    yidx = (np.arange(17) - 1) % 16
    xh = xb[:, zidx][:, :, yidx][:, :, :, yidx].reshape(CI, XH_LEN)
    buf = np.zeros((P, XH2_W), np.float32)
    buf[:CI, :XH_LEN] = xh
    buf[CI:, ZPL:ZPL + XH_LEN] = xh
    return buf


def _unshard(results) -> np.ndarray:
    out = np.empty((B, CO, 32, 32, 32), np.float32)
    for core in range(N_CORES):
        b, zhalf = core // 2, core % 2
        # free dims: (pair, pz, mzp, my, mx); partitions: (py, px, co)
        v = results[core]["out"].reshape(2, 2, CO, 4, 2, 2, 16, 16)
        # -> [co, (pair,mzp,pz), (my,py), (mx,px)]
        blk = v.transpose(2, 3, 5, 4, 6, 0, 7, 1).reshape(CO, 16, 32, 32)
        out[b, :, 16 * zhalf:16 * zhalf + 16] = blk
    return out


def kernel(x: np.ndarray, weight: np.ndarray) -> np.ndarray:
    import time
    from concourse.bass_utils import run_bass_kernel_spmd

    x = np.ascontiguousarray(np.asarray(x), dtype=np.float32)
    weight = np.ascontiguousarray(np.asarray(weight), dtype=np.float32)

    wt_host = _prep_weights(weight)
    in_maps = []
    for core in range(N_CORES):
        b, zhalf = core // 2, core % 2
        in_maps.append({"xh2": _prep_x_shard(x[b], zhalf), "wt": wt_host})

    nc = _get_program()
    last_exc = None
    for attempt in range(3):
        try:
            res = run_bass_kernel_spmd(nc, in_maps, core_ids=list(range(N_CORES)))
            return _unshard(res.results)
        except Exception as ex:  # intermittent NRT device errors: retry
            last_exc = ex
            time.sleep(2.0)
    raise last_exc


# revision 13
# speedup vs baseline: 1.2627x; 1.0628x over previous
"""Trainium2 Bass kernel for ConvTranspose3dMPS (FFT-based reference).

Math: the reference computes out = ifftn(fftn(upsample(x)) * fftn(pad(w))).real
summed over in-channels, i.e. a *circular* 3x3x3 convolution of the 2x
repeat-upsampled input.  Polyphase-decomposing by output parity (pz,py,px)
collapses it to eight 2x2x2-tap circular convolutions on the original 16^3
grid with parity-combined weights:

    out[b,co, 2m+p] = sum_{ci, t in {0,1}^3} We[p,t,co,ci] * x[b,ci,(m-t) mod 16]

where (per axis)  We uses  A[p=0] = [[1,0,0],[0,1,1]],  A[p=1] = [[1,1,0],[0,0,1]]
applied to the 3 kernel taps.  That is 32 matmuls of [K=128,M=128]x[128,512]
per (batch, z-half) shard — a perfect fit for the 128x128 PE array.

Sharding: 8 cores = batch(4) x z-half(2).  Each core receives:
  xh2 [128, 2890] f32 : halo-padded x slab (ci, 9x17x17 flat), duplicated on
                        partitions 64..127 at +289 elements so one access
                        pattern reads tap tz=0 (lower) and tz=1 (upper).
  wt  [128, 1024] f32 : parity-combined weights, [k=(tz,ci), (g,q)*128+m],
                        m = 32*(2*py+px) + co, g = pz, q = (ty,tx).
returns out [128, 4096] f32 : [32*(2py+px)+co, pair*1024 + pz*512 + mzp*256
                              + my*16 + mx]   (mz = 2*pair + mzp).

Kernel structure (per core): weights DMA (ACT ring) + x in 2 column-chunks
(SP ring); 8 warm-up matmuls on the weight tile ramp the PE HAM clock while
x streams in; 32 f32r matmuls (4 accumulating per PSUM bank) with 3-free-dim
strided rhs APs; PSUM evacuated by DVE/ACT copies; output DMAed in 2 chunks.
"""

import numpy as np

P = 128
B, CI, CO, NZ = 4, 64, 32, 16
ZPL = 17 * 17            # elements per halo z-plane
XH_LEN = 9 * ZPL         # 2601
XH2_W = XH_LEN + ZPL     # 2890 (upper-partition copy shifted +289)
WT_W = 8 * 128           # 1024
OUT_W = 2 * 8 * 256      # 4096
XCHUNK = 5 * ZPL         # 1445: column split point between mz pairs 0-1 / 2-3
N_CORES = 8
N_WARMUP = 5

_CACHE = {}


def _build_program():
    import concourse.bacc as bacc
    import concourse.tile as tile
    import concourse.mybir as mybir

    f32 = mybir.dt.float32
    f32r = mybir.dt.float32r

    nc = bacc.Bacc("TRN2", target_bir_lowering=False, debug=False)
    xh2_d = nc.declare_dram_parameter("xh2", [P, XH2_W], f32r, isOutput=False)
    wt_d = nc.declare_dram_parameter("wt", [P, WT_W], f32r, isOutput=False)
    out_d = nc.declare_dram_parameter("out", [P, OUT_W], f32, isOutput=True)

    with tile.TileContext(nc) as tc:
        with (
            tc.tile_pool(name="io", bufs=1) as io_pool,
            tc.tile_pool(name="ps", bufs=8, space="PSUM") as ps_pool,
        ):
            xh2 = io_pool.tile([P, XH2_W], f32r, tag="xh2")
            wt = io_pool.tile([P, WT_W], f32r, tag="wt")
            ob = io_pool.tile([P, OUT_W], f32, tag="ob")
            # all input DMAs on the SP HWDGE ring: FIFO per-engine ordering
            # guarantees wt lands first, then x chunk 1, then chunk 2
            nc.sync.dma_start(wt[:], wt_d[:])
            nc.sync.dma_start(xh2[:, 0:XCHUNK], xh2_d[:, 0:XCHUNK])
            nc.sync.dma_start(xh2[:, XCHUNK:XH2_W], xh2_d[:, XCHUNK:XH2_W])

            # PE warm-up on the weight tile (only needs wt; runs during x DMA)
            wu = ps_pool.tile([P, 512], f32, tag="wu", name="wu", bufs=1)
            for i in range(N_WARMUP):
                nc.tensor.matmul(wu[:], wt[:, 0:128], wt[:, 0:512],
                                 start=True, stop=True)

            def rhs_ap(pair, ty, tx):
                base = (2 * pair + 1) * ZPL + (1 - ty) * 17 + (1 - tx)
                return (
                    xh2[:, base:base + 578]
                    .rearrange("p (a b) -> p a b", a=2, b=289)[:, :, 0:272]
                    .rearrange("p a (c d) -> p a c d", c=16, d=17)[:, :, :, 0:16]
                )

            n_evac = 0
            for blk in range(2):               # mz pairs [2*blk, 2*blk+1]
                psums = [
                    ps_pool.tile([P, 512], f32, tag="ps", name=f"ps_{blk}_{i}", bufs=7)
                    for i in range(4)          # (pair_in_blk, g)
                ]
                for q in range(4):
                    ty, tx = q >> 1, q & 1
                    for g in range(2):
                        lhsT = wt[:, (g * 4 + q) * 128:(g * 4 + q + 1) * 128]
                        for pb in range(2):
                            pair = 2 * blk + pb
                            nc.tensor.matmul(
                                psums[2 * pb + g][:], lhsT, rhs_ap(pair, ty, tx),
                                start=(q == 0), stop=(q == 3),
                            )
                for pb in range(2):
                    pair = 2 * blk + pb
                    # two evacs per pair in parallel on DVE + ACT
                    nc.vector.tensor_copy(
                        ob[:, pair * 1024:pair * 1024 + 512], psums[2 * pb][:])
                    nc.scalar.copy(
                        ob[:, pair * 1024 + 512:pair * 1024 + 1024],
                        psums[2 * pb + 1][:])
                    # stream this pair's quarter of the output immediately
                    quarter = ob[:, pair * 1024:(pair + 1) * 1024]
                    quarter_d = out_d[:, pair * 1024:(pair + 1) * 1024]
                    if pb == 0:
                        nc.scalar.dma_start(quarter_d, quarter)
                    else:
                        nc.sync.dma_start(quarter_d, quarter)

    nc.compile()
    return nc


def _get_program():
    if "nc" not in _CACHE:
        _CACHE["nc"] = _build_program()
    return _CACHE["nc"]


# Per-axis tap combination: A[p][t, d]
_A = np.array(
    [[[1, 0, 0], [0, 1, 1]],
     [[1, 1, 0], [0, 0, 1]]], dtype=np.float32)


def _prep_weights(weight: np.ndarray) -> np.ndarray:
    # We[pz,py,px,tz,ty,tx,co,ci]
    We = np.einsum("ptd,que,rvf,oidef->pqrtuvoi", _A, _A, _A,
                   weight.astype(np.float32), optimize=True)
    WT = np.zeros((2, 4, P, P), np.float32)
    for g in range(2):
        for q in range(4):
            ty, tx = q >> 1, q & 1
            for tz in range(2):
                for pp in range(4):
                    py, px = pp >> 1, pp & 1
                    WT[g, q, 64 * tz:64 * tz + 64, 32 * pp:32 * pp + 32] = (
                        We[g, py, px, tz, ty, tx].T
                    )
    # DRAM layout [k, (g,q)*128 + m]
    return np.ascontiguousarray(WT.transpose(2, 0, 1, 3).reshape(P, WT_W))


def _prep_x_shard(xb: np.ndarray, zhalf: int) -> np.ndarray:
    # xb: [CI, 16, 16, 16] one batch; build halo slab + partition duplication
    z0 = 8 * zhalf
    zidx = (z0 - 1 + np.arange(9)) % 16
    yidx = (np.arange(17) - 1) % 16
    xh = xb[:, zidx][:, :, yidx][:, :, :, yidx].reshape(CI, XH_LEN)
    buf = np.zeros((P, XH2_W), np.float32)
    buf[:CI, :XH_LEN] = xh
    buf[CI:, ZPL:ZPL + XH_LEN] = xh
    return buf


def _unshard(results) -> np.ndarray:
    out = np.empty((B, CO, 32, 32, 32), np.float32)
    for core in range(N_CORES):
        b, zhalf = core // 2, core % 2
        # free dims: (pair, pz, mzp, my, mx); partitions: (py, px, co)
        v = results[core]["out"].reshape(2, 2, CO, 4, 2, 2, 16, 16)
        # -> [co, (pair,mzp,pz), (my,py), (mx,px)]
        blk = v.transpose(2, 3, 5, 4, 6, 0, 7, 1).reshape(CO, 16, 32, 32)
        out[b, :, 16 * zhalf:16 * zhalf + 16] = blk
    return out


def kernel(x: np.ndarray, weight: np.ndarray) -> np.ndarray:
    import time
    from concourse.bass_utils import run_bass_kernel_spmd

    x = np.ascontiguousarray(np.asarray(x), dtype=np.float32)
    weight = np.ascontiguousarray(np.asarray(weight), dtype=np.float32)

    wt_host = _prep_weights(weight)
    in_maps = []
    for core in range(N_CORES):
        b, zhalf = core // 2, core % 2
        in_maps.append({"xh2": _prep_x_shard(x[b], zhalf), "wt": wt_host})

    nc = _get_program()
    last_exc = None
    for attempt in range(3):
        try:
            res = run_bass_kernel_spmd(nc, in_maps, core_ids=list(range(N_CORES)))
            return _unshard(res.results)
        except Exception as ex:  # intermittent NRT device errors: retry
            last_exc = ex
            time.sleep(2.0)
    raise last_exc
